# revision 31
# baseline (speedup 1.0000x reference)
"""Multi-head causal attention (B=4, S=2048, H=1024, 16 heads) on 8 TRN2 cores.

Sharding: batch (4) x head-group (2x8 heads) -> 8 cores. Each core computes,
for one batch and 8 heads: QKV projections, causal softmax attention, and its
partial output projection. Host sums the two head-group partials per batch and
adds the output bias (bo + bv @ Wo.T; the value bias commutes through softmax
attention, and the key bias shifts every softmax row by a constant, so neither
is applied on device).

Device layout (per core):
  qT/kT: [128, 2048] bf16 per head pair (two 64-dim heads stacked on
         partitions); scoresT = kT.T @ qT runs as row-tiled K=64 matmul pairs
         on the two 64-row PE tiles.
  vextH: bf16 [128 s, 8 head, 16 kt, 65] with column 64 = 1.0, so each PV
         matmul (M=65) accumulates PV and the softmax denominator (PSUM row
         64) for free (no separate ones-vector matmuls).
  scoresT groups [128, 2(kt), 512(q)] in PSUM -> exp on ScalarE -> probsT
  bf16 (valid scores reach ~292, far beyond fp8 range); causal triangle
  applied post-exp by a DVE multiply on diagonal blocks; exp clipped to the
  causal-valid q range on the o=256 diagonal group. 1/l on DVE reciprocal,
  partition-broadcast on GpSimd, applied by DVE multiplies. Final
  projection y = out @ WoT accumulates over the 4 pairs.

Projections and the previous slab's output projection are software-pipelined
with attention: a fill queue of matmul-chain items is drained between
attention groups so the PE never idles (keeps the HAM clock gate at 2.4 GHz).
"""

import sys

sys.path.insert(0, "/opt/trn_rl_repo")

import math
from contextlib import ExitStack

import numpy as np
import ml_dtypes

import concourse.bass as bass
import concourse.mybir as mybir
from concourse import bacc
from concourse.tile import TileContext
from concourse.bass_utils import run_bass_kernel_spmd

BF16 = mybir.dt.bfloat16
F32 = mybir.dt.float32
AF = mybir.ActivationFunctionType
ALU = mybir.AluOpType

B, S, H = 4, 2048, 1024
NH, DH = 16, 64
O = 512          # per-core output dim of q/k/v projections (8 heads x 64)
NPAIR = 4        # head pairs per core
NSLAB = 4        # q slabs of 512
NST = 16         # s-tiles of 128
MASK_FILL = -8.0e5  # pre-scale (x0.125) additive mask for padded keys

_BUILT = {}


def _build(general_mask: bool):
    if general_mask in _BUILT:
        return _BUILT[general_mask]

    nc = bacc.Bacc("TRN2", target_bir_lowering=False, debug=False)

    xq_d = [nc.dram_tensor(f"xq{i}", [H, 512], BF16, kind="ExternalInput") for i in range(4)]
    xk_d = [nc.dram_tensor(f"xk{i}", [H, 512], BF16, kind="ExternalInput") for i in range(4)]
    xv_d = [nc.dram_tensor(f"xv{i}", [H, 512], BF16, kind="ExternalInput") for i in range(4)]
    wqT = nc.dram_tensor("wqT", [H, O], BF16, kind="ExternalInput")
    wkT = nc.dram_tensor("wkT", [H, O], BF16, kind="ExternalInput")
    wvT = nc.dram_tensor("wvT", [H, O], BF16, kind="ExternalInput")
    woT = nc.dram_tensor("woT", [O, H], BF16, kind="ExternalInput")
    bqc = nc.dram_tensor("bqc", [128, 4], F32, kind="ExternalInput")
    tri = nc.dram_tensor("tri", [128, 896], BF16, kind="ExternalInput")
    mb = nc.dram_tensor("mb", [1, S], BF16, kind="ExternalInput")
    y = nc.dram_tensor("y", [S, H], F32, kind="ExternalOutput")

    with TileContext(nc) as tc, ExitStack() as ctx:
        P = lambda name, bufs, **kw: ctx.enter_context(
            tc.tile_pool(name=name, bufs=bufs, **kw)
        )
        wp = P("wp", 1)
        xp = P("xp", 6)
        qk = P("qk", 1)
        vx = P("vx", 1)
        pb = P("pb", 6)
        ob = P("ob", 8)
        ls = P("ls", 2)
        rxp = P("rxp", 4)
        ys = P("ys", 4)

        scp = P("scp", 2, space="PSUM")    # scoresT groups [128,2,512] (4 banks)
        ovp = P("ovp", 3, space="PSUM")    # PV+denominator [65,512] (3 banks)
        ppp = P("ppp", 1, space="PSUM")    # projection chains + yproj (1 bank)

        wq_sb = wp.tile([128, 8, O], BF16, tag="wq", name="wq")
        wk_sb = wp.tile([128, 8, O], BF16, tag="wk", name="wk")
        wv_sb = wp.tile([128, 8, O], BF16, tag="wv", name="wv")
        wo_sb = wp.tile([128, 4, H], BF16, tag="wo", name="wo")
        bq_sb = wp.tile([128, 4], F32, tag="bq", name="bq")
        tri_sb = wp.tile([128, 896], BF16, tag="tri", name="tri")
        nc.sync.dma_start(wq_sb[:], wqT.rearrange("(po pi) o -> pi po o", pi=128))
        nc.sync.dma_start(bq_sb[:], bqc[:, :])
        nc.sync.dma_start(wk_sb[:], wkT.rearrange("(po pi) o -> pi po o", pi=128))
        nc.sync.dma_start(wv_sb[:], wvT.rearrange("(po pi) o -> pi po o", pi=128))
        nc.sync.dma_start(wo_sb[:], woT.rearrange("(po pi) j -> pi po j", pi=128))
        nc.sync.dma_start(tri_sb[:], tri[:, :])
        if general_mask:
            mb_sb = wp.tile([1, S], BF16, tag="mb", name="mb")
            nc.sync.dma_start(mb_sb[:], mb[:, :])
            ones_row = wp.tile([1, 512], BF16, tag="onr", name="onr")
            nc.gpsimd.memset(ones_row[:], 1.0)

        # vext[s, head, ktpair, kt, 0:64] = v (fp8), [..., 64] = 1.0; cols
        # 65-79 pad the kt stride to 80 B (DoubleRow needs step % 16 == 0).
        # Off-diagonal (unmasked) groups use it with DoubleRow; diagonal
        # groups use the bf16 copy vextH (bf16 probs so the DVE tri-multiply
        # stays on a proven dtype).
        vext = vx.tile([128, 8, 8, 2, 80], FP8, tag="vext", name="vext")
        nc.gpsimd.memset(vext[:], 1.0)
        vextH = vx.tile([128, 8, NST, 65], BF16, tag="vextH", name="vextH")
        nc.gpsimd.memset(vextH[:], 1.0)

        # diagonal pb slots are partially written when exp is clipped to the
        # causal range; memset once so stale bytes are finite (tri-mult
        # zeroes them)
        pb_init = [pb.tile([128, 2, 512], BF16, tag="pbh", name="pbh") for _ in range(6)]
        for t in pb_init:
            nc.gpsimd.memset(t[:], 0.0)

        # exp bias -ln16: probs come out as p/16 so fp8 e4m3 (max 448) never
        # overflows; the factor cancels in the l normalization
        eb_sb = wp.tile([128, 1], F32, tag="eb", name="eb")
        nc.gpsimd.memset(eb_sb[:], -2.772588722239781)

        qT_sb = [qk.tile([128, S], BF16, tag=f"qT{p}", name=f"qT{p}") for p in range(NPAIR)]
        kT_sb = [qk.tile([128, S], BF16, tag=f"kT{p}", name=f"kT{p}") for p in range(NPAIR)]

        x_tiles = {}

        def emit_stage_dma(sl):
            for nmkey, dram in (("q", xq_d[sl]), ("k", xk_d[sl]), ("v", xv_d[sl])):
                t = xp.tile([128, 8, 512], BF16, tag="xp", name=f"x{nmkey}{sl}")
                nc.sync.dma_start(t[:], dram.rearrange("(po pi) s -> pi po s", pi=128))
                x_tiles[(nmkey, sl)] = t

        def qk_chain_items(kind, pair, sl):
            w_sb = wq_sb if kind == "q" else wk_sb
            dst = qT_sb[pair] if kind == "q" else kT_sb[pair]
            x_t = x_tiles[(kind, sl)]
            hold = {}

            def part1():
                ps = ppp.tile([128, 512], F32, tag="pp", name="pp")
                hold["ps"] = ps
                for ic in range(4):
                    nc.tensor.matmul(
                        ps[:],
                        w_sb[:, ic, 128 * pair : 128 * pair + 128],
                        x_t[:, ic, :],
                        start=(ic == 0),
                        stop=False,
                    )

            def part2():
                ps = hold["ps"]
                for ic in range(4, 8):
                    nc.tensor.matmul(
                        ps[:],
                        w_sb[:, ic, 128 * pair : 128 * pair + 128],
                        x_t[:, ic, :],
                        start=False,
                        stop=(ic == 7),
                    )
                cols = slice(512 * sl, 512 * sl + 512)
                if kind == "q":
                    nc.vector.tensor_scalar_add(
                        dst[:, cols], ps[:], bq_sb[:, pair : pair + 1]
                    )
                else:
                    nc.vector.tensor_copy(dst[:, cols], ps[:])

            return [part1, part2]

        def v_chain_items(st):
            sl = st // 4
            x_t = x_tiles[("v", sl)]
            c0 = 128 * (st % 4)
            hold = {}

            def part1():
                ps = ppp.tile([128, 8, 64], F32, tag="pp", name="pp")
                hold["ps"] = ps
                for ic in range(4):
                    nc.tensor.matmul(
                        ps[:],
                        x_t[:, ic, c0 : c0 + 128],
                        wv_sb[:, ic, :],
                        start=(ic == 0),
                        stop=False,
                    )

            def part2():
                ps = hold["ps"]
                for ic in range(4, 8):
                    nc.tensor.matmul(
                        ps[:],
                        x_t[:, ic, c0 : c0 + 128],
                        wv_sb[:, ic, :],
                        start=False,
                        stop=(ic == 7),
                    )
                nc.vector.tensor_copy(vext[:, :, st // 2, st % 2, 0:64], ps[:, :, :])
                nc.vector.tensor_copy(vextH[:, :, st, 0:64], ps[:, :, :])

            return [part1, part2]

        def stage_items(sl):
            # stage-3 q/k chains for pair p are only needed once slab-3
            # attention reaches pair p: label them sl + p/10 so they act as
            # PE fill inside slab 3 (which otherwise runs dry and cold)
            items = []
            for pair in range(NPAIR):
                lbl = sl + pair / 10
                items += [(lbl, f) for f in qk_chain_items("q", pair, sl)]
                items += [(lbl, f) for f in qk_chain_items("k", pair, sl)]
            vlbl = sl + 0.04 if sl >= 1 else 0.0
            for st in range(4 * sl, 4 * sl + 4):
                items += [(vlbl, f) for f in v_chain_items(st)]
            return items

        fill_queue = []

        def drain_fill(n, reserve=0, ceiling=99.0):
            # pop the first item whose label is <= ceiling: items labeled for
            # a later slab are held back so they can fill that slab's PE gaps
            for _ in range(n):
                if len(fill_queue) <= reserve:
                    return
                for idx, (lbl, f) in enumerate(fill_queue):
                    if lbl <= ceiling:
                        fill_queue.pop(idx)
                        f()
                        break
                else:
                    return

        def flush_fill(upto):
            rest = []
            for lbl, f in fill_queue:
                if lbl <= upto:
                    f()
                else:
                    rest.append((lbl, f))
            fill_queue[:] = rest

        def yproj_items(slab, o_tiles):
            items = []
            for st in range(4):
                srow = 512 * slab + 128 * st

                def item(st=st, srow=srow):
                    for jsl in range(2):
                        y_ps = ppp.tile([128, 512], F32, tag="pp", name="pp")
                        for pair in range(NPAIR):
                            nc.tensor.matmul(
                                y_ps[:],
                                o_tiles[pair][:, 128 * st : 128 * st + 128],
                                wo_sb[:, pair, 512 * jsl : 512 * jsl + 512],
                                start=(pair == 0),
                                stop=(pair == NPAIR - 1),
                            )
                        ysb = ys.tile([128, 512], F32, tag="ys", name="ys")
                        nc.vector.tensor_copy(ysb[:], y_ps[:])
                        nc.sync.dma_start(
                            y[srow : srow + 128, 512 * jsl : 512 * jsl + 512], ysb[:]
                        )

                items.append(item)
            return items

        def emit_attention_slab(slab):
            flush_fill(slab + 0.001)
            q0 = 512 * slab
            n_kt = 4 * (slab + 1)
            n_g = n_kt // 2
            o_tiles = []
            for pair in range(NPAIR):
                flush_fill(slab + pair / 10 + 0.001)
                psA = ovp.tile([65, 512], F32, tag="ov", name="ov")
                psB = ovp.tile([65, 512], F32, tag="ov", name="ov")
                for g in range(n_g):
                    if slab >= 1 and g == 2 * slab:
                        flush_fill(slab + 0.045)
                    kts = (2 * g, 2 * g + 1)
                    scA = scp.tile([128, 2, 512], F32, tag="sc", name="sc")
                    scB = scp.tile([128, 2, 512], F32, tag="sc", name="sc")
                    for j, kt in enumerate(kts):
                        for hh, sc in ((0, scA), (1, scB)):
                            r0 = 64 * hh
                            nc.tensor.matmul(
                                sc[:, j, :],
                                kT_sb[pair][r0 : r0 + 64, 128 * kt : 128 * kt + 128],
                                qT_sb[pair][r0 : r0 + 64, q0 : q0 + 512],
                                start=True,
                                stop=not general_mask,
                            )
                            if general_mask:
                                nc.tensor.matmul(
                                    sc[:, j, :],
                                    mb_sb[0:1, 128 * kt : 128 * kt + 128],
                                    ones_row[0:1, :],
                                    start=False,
                                    stop=True,
                                )
                    drain_fill(1, reserve=4)
                    diag = g >= 2 * slab
                    # exp emits probs/16 (bias -ln16) so fp8 e4m3 never
                    # overflows; the 1/16 cancels in the l normalization
                    if diag:
                        pbA = pb.tile([128, 2, 512], BF16, tag="pbh", name="pbh")
                        pbB = pb.tile([128, 2, 512], BF16, tag="pbh", name="pbh")
                    else:
                        pbA = pb.tile([128, 2, 512], FP8P, tag="pb8", name="pb8")
                        pbB = pb.tile([128, 2, 512], FP8P, tag="pb8", name="pb8")
                    # diagonal group at o=256: cols < 256 are fully masked for
                    # both kts; skip them in exp (tri-mult zeroes stale bytes)
                    c0 = 256 if (slab > 0 and g == 2 * slab + 1) else 0
                    nc.scalar.activation(
                        pbA[:, :, c0:], scA[:, :, c0:], AF.Exp, scale=0.125, bias=eb_sb[:, 0:1]
                    )
                    nc.scalar.activation(
                        pbB[:, :, c0:], scB[:, :, c0:], AF.Exp, scale=0.125, bias=eb_sb[:, 0:1]
                    )
                    if diag:
                        for j, kt in enumerate(kts):
                            o = 128 * (kt - 4 * slab)
                            w = o + 128
                            for p_t in (pbA, pbB):
                                nc.vector.tensor_tensor(
                                    p_t[:, j, 0:w],
                                    p_t[:, j, 0:w],
                                    tri_sb[:, 384 - o : 384 - o + w],
                                    ALU.mult,
                                )
                    drain_fill(1, reserve=4)
                    if diag:
                        for j, kt in enumerate(kts):
                            for hh, p_t, ps_h in ((0, pbA, psA), (1, pbB, psB)):
                                nc.tensor.matmul(
                                    ps_h[0:65, :],
                                    vextH[:, 2 * pair + hh, kt, 0:65],
                                    p_t[:, j, :],
                                    start=(g == 0 and j == 0),
                                    stop=(g == n_g - 1 and j == 1),
                                )
                    else:
                        for hh, p_t, ps_h in ((0, pbA, psA), (1, pbB, psB)):
                            nc.tensor.matmul(
                                ps_h[0:65, :],
                                vext[:, 2 * pair + hh, g, 0:2, 0:65],
                                p_t[:, :, :],
                                start=(g == 0),
                                stop=False,
                                perf_mode=DR,
                            )
                # normalization off the PE critical path: evacuate PV+l,
                # 1/l on DVE, broadcast on GpSimd, multiply on DVE
                poA = ls.tile([65, 512], BF16, tag="poA", name="poA")
                poB = ls.tile([65, 512], BF16, tag="poB", name="poB")
                nc.any.tensor_copy(poA[:], psA[:])
                nc.any.tensor_copy(poB[:], psB[:])
                lsb = ls.tile([33, 512], BF16, tag="lsb", name="lsb")
                nc.gpsimd.memset(lsb[:], 1.0)
                nc.vector.tensor_copy(lsb[0:1, :], poA[64:65, :])
                nc.vector.tensor_copy(lsb[32:33, :], poB[64:65, :])
                lrec = ls.tile([33, 512], BF16, tag="lrec", name="lrec")
                with nc.allow_low_precision(reason="1/l in bf16: 0.4% scale err ok"):
                    nc.vector.reciprocal(lrec[:], lsb[:])
                lrB0 = ls.tile([1, 512], BF16, tag="lrB0", name="lrB0")
                nc.vector.tensor_copy(lrB0[0:1, :], lrec[32:33, :])
                rxA = rxp.tile([128, 512], BF16, tag="rx", name="rx")
                rxB = rxp.tile([128, 512], BF16, tag="rx", name="rx")
                nc.gpsimd.partition_broadcast(rxA[:], lrec[0:1, :])
                nc.gpsimd.partition_broadcast(rxB[:], lrB0[0:1, :])
                o_sb = ob.tile([128, 512], BF16, tag="ob", name="ob")
                nc.vector.tensor_tensor(
                    o_sb[0:64, :], poA[0:64, :], rxA[0:64, :], ALU.mult
                )
                nc.vector.tensor_copy(o_sb[64:128, :], poB[0:64, :])
                nc.vector.tensor_tensor(
                    o_sb[64:128, :], o_sb[64:128, :], rxB[64:128, :], ALU.mult
                )
                o_tiles.append(o_sb)
                drain_fill(4, ceiling=slab + 1)

            # defer this slab's output projection into the next slab's fill
            # queue (o_sb lifetime: ob bufs=8 = two slabs)
            yitems = yproj_items(slab, o_tiles)
            if slab + 1 < NSLAB:
                pos = min(8, len(fill_queue))
                fill_queue[pos:pos] = [(slab + 1.05, f) for f in yitems]
            else:
                for f in yitems:
                    f()

        emit_stage_dma(0)
        emit_stage_dma(1)
        fill_queue.extend(stage_items(0))
        drain_fill(len(fill_queue), ceiling=0.0)
        for slab in range(NSLAB):
            if slab + 2 < NSLAB:
                emit_stage_dma(slab + 2)
            if slab + 1 < NSLAB:
                fill_queue.extend(stage_items(slab + 1))
            emit_attention_slab(slab)
        flush_fill(NSLAB)

    nc.compile()
    _BUILT[general_mask] = nc
    return nc


def _prep_core(query, key, value, mask, Wq, bq, Wk, Wv, Wo, core):
    b, hg = core // 2, core % 2
    o_sl = slice(hg * O, hg * O + O)
    bf = ml_dtypes.bfloat16

    tri = np.zeros((128, 896), dtype=np.float32)
    j = np.arange(896)[None, :]
    kk = np.arange(128)[:, None]
    tri[(j - 384) >= kk] = 1.0

    mrow = np.where(mask[b] > 0, 0.0, MASK_FILL).astype(np.float32)

    xqT = np.ascontiguousarray(query[b].T).astype(bf)
    xkT = np.ascontiguousarray(key[b].T).astype(bf)
    xvT = np.ascontiguousarray(value[b].T).astype(bf)

    d = {
        "wqT": np.ascontiguousarray(Wq[o_sl].T).astype(bf),
        "wkT": np.ascontiguousarray(Wk[o_sl].T).astype(bf),
        "wvT": np.ascontiguousarray(Wv[o_sl].T).astype(bf),
        "woT": np.ascontiguousarray(Wo[:, o_sl].T).astype(bf),
        "bqc": np.ascontiguousarray(bq[o_sl].reshape(4, 128).T).astype(np.float32),
        "tri": tri.astype(bf),
        "mb": mrow.reshape(1, S).astype(bf),
    }
    for i in range(4):
        cs = slice(512 * i, 512 * i + 512)
        d[f"xq{i}"] = np.ascontiguousarray(xqT[:, cs])
        d[f"xk{i}"] = np.ascontiguousarray(xkT[:, cs])
        d[f"xv{i}"] = np.ascontiguousarray(xvT[:, cs])
    return d


def kernel(query, key, value, mask, Wq, bq, Wk, bk, Wv, bv, Wo, bo, _trace=False):
    general_mask = bool(np.any(np.asarray(mask) <= 0))
    nc = _build(general_mask)
    in_maps = [
        _prep_core(query, key, value, mask, Wq, bq, Wk, Wv, Wo, c)
        for c in range(8)
    ]
    res = run_bass_kernel_spmd(
        nc, in_maps, core_ids=list(range(8)), trace=_trace,
        trace_cores=list(range(8)) if _trace else None,
    )
    parts = np.stack([res.results[c]["y"] for c in range(8)])  # [8, S, H]
    # bk shifts every softmax row by a per-query constant (cancels exactly);
    # bv commutes through attention: its contribution is the constant bv @ Wo.T
    bias = np.asarray(bo) + np.asarray(bv) @ np.asarray(Wo).T
    out = parts[0::2] + parts[1::2] + bias[None, None, :]
    if _trace:
        kernel.last_results = res
    return out.astype(np.float32)
